# revision 33
# baseline (speedup 1.0000x reference)
"""Trainium2 Bass kernel for nn_Matposer_51007031608225.

Key algebraic insight: the reference computes fmap = einsum('bld,ble->bde')
(a [512,300,300] bmm) but then keeps only diagonal(fmap, axis1=0, axis2=1),
i.e. fmap[k,k,:] for k < 300.  So per batch-index k only

    diagT[k, e] = sum_l e2[k,l,k] * e1[k,l,e]
                = sum_l a_k[l] * (scale*emb1[x1[k,l],e] + pe[l,e])
    a_k[l]      = scale*emb2[x2[k,l],k] + pe[l,k]

is needed — a [300x512]@[512] matvec per k instead of the full bmm.  The
dominant cost becomes gathering 300*512 embedding rows (~190 MB), which is
data-parallel over k across the 8 cores.  The tiny [300,300] MLP head couples
all k (contraction over k before the ReLU), so it runs as a second, tiny
single-core kernel after the host concatenates the per-core diagonal slices
(the "all-gather" of the sharding hint).

Phase 1 (SPMD x8, k-sharded 38 per core, ~19 pipelined 2-k chunks):
  - dma_gather emb1 rows (padded to 320 f32 for the 256B-alignment rule)
  - dma_gather per-core channel-sliced emb2 (32000x64 slabs), extract the
    single needed channel with static strided copies
  - per k: 4 fp32 matmuls (lhsT = scaled a-column [128,1], rhs = gathered
    rows [128,300]) accumulating diagG_k = G_k^T (scale*a_k) in PSUM
  - batched pe-term: diagPE = A^T pe via 4 matmuls (M=38)
Phase 2 (tiny, 1 core): after the host concatenates the per-core diagonal
  slices (the "all-gather" of the sharding hint): diagT = diagG + diagPE;
  h = relu(w1T^T diagT + b1); logits = h^T w2T + b2; softmax over the
  4-wide free dim.
A FUSED single-launch variant (in-kernel AllGather + redundant head on all
cores) is implemented too, but each small collective costs ~15us fixed, so
the two-launch version is faster on device time.
"""

import numpy as np
from contextlib import ExitStack

import concourse.bass as bass
import concourse.bacc as bacc
import concourse.tile as tile
import concourse.mybir as mybir
from concourse.bass_utils import run_bass_kernel_spmd

F32 = mybir.dt.float32
I16 = mybir.dt.int16

D = 300          # d_model
L = 512          # sequence length
V = 32000        # vocab
OUT = 4
NCORES = 8
NK = 38          # k's per core (8*38 = 304 >= 300)
EP = 320         # padded emb1 row (f32), 1280B (mult of 256B)
E2P = 64         # padded per-core emb2 channel slab (f32), 256B
CHUNK_SIZES = [2] * 19          # k's per gather chunk; sums to NK
SCALE = float(np.sqrt(np.float32(D)))


# ---------------------------------------------------------------- phase 1

def _build_phase1(fused=True):
    nc = bacc.Bacc("TRN2", target_bir_lowering=False, debug=False,
                   num_devices=NCORES)

    emb1p = nc.dram_tensor("emb1p", [V, EP], F32, kind="ExternalInput").ap()
    emb2sl = nc.dram_tensor("emb2sl", [V, E2P], F32, kind="ExternalInput").ap()
    x1w_d = nc.dram_tensor("x1w", [128, NK * 32], I16, kind="ExternalInput").ap()
    x2w_d = nc.dram_tensor("x2w", [128, NK * 32], I16, kind="ExternalInput").ap()
    pe4_d = nc.dram_tensor("pe4", [128, 4 * D], F32, kind="ExternalInput").ap()
    pec_d = nc.dram_tensor("pec", [128, NK * 4], F32, kind="ExternalInput").ap()
    if fused:
        w1T_d = nc.dram_tensor("w1T", [D, D], F32, kind="ExternalInput").ap()
        b1_d = nc.dram_tensor("b1c", [D, 1], F32, kind="ExternalInput").ap()
        w2T_d = nc.dram_tensor("w2T", [D, OUT], F32, kind="ExternalInput").ap()
        b2_d = nc.dram_tensor("b2b", [128, OUT], F32, kind="ExternalInput").ap()
        out_d = nc.dram_tensor("out", [D, OUT], F32, kind="ExternalOutput").ap()
        dlocG = nc.dram_tensor("dlocG", [1, NK * D], F32).ap()
        dlocPE = nc.dram_tensor("dlocPE", [1, NK * D], F32).ap()
        dallG = nc.dram_tensor("dallG", [NCORES, NK * D], F32).ap()
        dallPE = nc.dram_tensor("dallPE", [NCORES, NK * D], F32).ap()
    else:
        diagG_d = nc.dram_tensor("diagG", [1, NK * D], F32, kind="ExternalOutput").ap()
        diagPE_d = nc.dram_tensor("diagPE", [NK, D], F32, kind="ExternalOutput").ap()

    with tile.TileContext(nc) as tc, ExitStack() as ctx:
        cpool = ctx.enter_context(tc.tile_pool(name="consts", bufs=1))
        g1pool = ctx.enter_context(tc.tile_pool(name="g1", bufs=8))
        g2pool = ctx.enter_context(tc.tile_pool(name="g2", bufs=8))
        spool = ctx.enter_context(tc.tile_pool(name="small", bufs=1))
        ps_ctx = ctx.enter_context(ExitStack())
        pk_ps = ps_ctx.enter_context(tc.tile_pool(name="pk", bufs=6, space="PSUM"))
        pe_ps = ps_ctx.enter_context(tc.tile_pool(name="ppe", bufs=1, space="PSUM"))

        x1w = cpool.tile([128, NK * 32], I16)
        nc.sync.dma_start(x1w[:], x1w_d[:])
        x2w = cpool.tile([128, NK * 32], I16)
        nc.sync.dma_start(x2w[:], x2w_d[:])
        pe4 = cpool.tile([128, 4 * D], F32)
        nc.sync.dma_start(pe4[:], pe4_d[:])
        pec = cpool.tile([128, NK * 4], F32)
        nc.sync.dma_start(pec[:], pec_d[:])

        preload = None
        if fused:
            # head weights don't depend on the gathers/collective — load early
            KC = [(0, 128), (128, 128), (256, 44)]
            w1tt, w2tt, b1tt = [], [], []
            b2t = cpool.tile([128, OUT], F32)
            nc.sync.dma_start(b2t[:], b2_d[:])
            for i, (k0, kn) in enumerate(KC):
                tw = cpool.tile([128, D], F32, tag=f"hw1{i}")
                nc.sync.dma_start(tw[:kn, :], w1T_d[k0:k0 + kn, :])
                w1tt.append(tw)
                t2 = cpool.tile([128, OUT], F32, tag=f"hw2{i}")
                nc.sync.dma_start(t2[:kn, :], w2T_d[k0:k0 + kn, :])
                w2tt.append(t2)
                tb = cpool.tile([128, 1], F32, tag=f"hb1{i}")
                nc.sync.dma_start(tb[:kn, :], b1_d[k0:k0 + kn, :])
                b1tt.append(tb)
            preload = (w1tt, w2tt, b1tt, b2t)

        a_raw = spool.tile([128, NK * 4], F32)
        a_full = spool.tile([128, NK * 4], F32)
        s_a = spool.tile([128, NK * 4], F32)
        stageG = spool.tile([1, NK * D], F32)

        off = 0
        for ch in CHUNK_SIZES:
            ni = ch * L
            # ---- emb1 row gather first (it gates the PE work)
            g1 = g1pool.tile([128, ch * 4 * EP], F32, tag="g1")
            nc.gpsimd.dma_gather(
                out_ap=g1[:].rearrange("p (c e) -> p c e", e=EP),
                in_ap=emb1p[:],
                idxs_ap=x1w[:, off * 32:(off + ch) * 32],
                num_idxs=ni,
                num_idxs_reg=ni,
                elem_size=EP,
                single_packet=False,
            )
            # ---- emb2 channel-slab gather for this chunk's k's
            g2 = g2pool.tile([128, ch * 4 * E2P], F32, tag="g2")
            nc.gpsimd.dma_gather(
                out_ap=g2[:].rearrange("p (c e) -> p c e", e=E2P),
                in_ap=emb2sl[:],
                idxs_ap=x2w[:, off * 32:(off + ch) * 32],
                num_idxs=ni,
                num_idxs_reg=ni,
                elem_size=E2P,
                single_packet=False,
            )
            g2v = g2[:].rearrange("p (c e) -> p c e", e=E2P)
            for kk in range(ch):
                klc = off + kk   # core-local k == channel in emb2sl
                nc.vector.tensor_copy(
                    a_raw[:, klc * 4:(klc + 1) * 4],
                    g2v[:, kk * 4:(kk + 1) * 4, klc],
                )
            # a_full = scale*a_raw + pe_cols ; s_a = scale*a_full
            cols = slice(off * 4, (off + ch) * 4)
            nc.vector.tensor_scalar_mul(a_full[:, cols], a_raw[:, cols], SCALE)
            nc.vector.tensor_tensor(
                out=a_full[:, cols], in0=a_full[:, cols], in1=pec[:, cols],
                op=mybir.AluOpType.add,
            )
            nc.vector.tensor_scalar_mul(s_a[:, cols], a_full[:, cols], SCALE)

            # ---- per-k matvec: diagG_k = G_k^T (scale * a_k)
            for kk in range(ch):
                klc = off + kk
                pk = pk_ps.tile([1, D], F32, tag="pk")
                for c in range(4):
                    nc.tensor.matmul(
                        out=pk[:],
                        lhsT=s_a[:, klc * 4 + c: klc * 4 + c + 1],
                        rhs=g1[:, (kk * 4 + c) * EP: (kk * 4 + c) * EP + D],
                        start=(c == 0),
                        stop=(c == 3),
                    )
                nc.any.tensor_copy(stageG[:1, klc * D:(klc + 1) * D], pk[:])
            off += ch

        # ---- batched pe term: diagPE = A^T pe  (A = a_full, [512, NK])
        ppe = pe_ps.tile([NK, D], F32)
        afv = a_full[:].rearrange("p (k c) -> p c k", c=4)
        for c in range(4):
            nc.tensor.matmul(
                out=ppe[:],
                lhsT=afv[:, c, :],
                rhs=pe4[:, c * D:(c + 1) * D],
                start=(c == 0),
                stop=(c == 3),
            )
        stagePE = spool.tile([NK, D], F32)
        nc.vector.tensor_copy(stagePE[:], ppe[:])

        ps_ctx.close()   # free phase-1 PSUM banks before the head allocates

        if not fused:
            nc.sync.dma_start(diagG_d[:], stageG[:])
            nc.sync.dma_start(diagPE_d[:], stagePE[:])
        else:
            nc.sync.dma_start(dlocG[:], stageG[:])
            nc.sync.dma_start(dlocPE[:], stagePE[:])
            nc.gpsimd.collective_compute(
                "AllGather", mybir.AluOpType.bypass,
                replica_groups=[list(range(NCORES))],
                ins=[dlocG[:]], outs=[dallG[:]],
            )
            nc.gpsimd.collective_compute(
                "AllGather", mybir.AluOpType.bypass,
                replica_groups=[list(range(NCORES))],
                ins=[dlocPE[:]], outs=[dallPE[:]],
            )
            dGv = dallG[:].rearrange("n (k e) -> (n k) e", e=D)
            dPEv = dallPE[:].rearrange("n (k e) -> (n k) e", e=D)
            _head(nc, tc, ctx, dGv, dPEv, None, None, None, None, out_d,
                  preload=preload)

    nc.compile()
    return nc


def _head(nc, tc, ctx, dG_d, dPE_d, w1T_d, b1_d, w2T_d, b2_d, out_d,
          preload=None):
    """The [300,300] MLP head + softmax, k on partitions in 3 chunks."""
    KC = [(0, 128), (128, 128), (256, 44)]
    pool = ctx.enter_context(tc.tile_pool(name="hd", bufs=1))
    psum = ctx.enter_context(tc.tile_pool(name="hdps", bufs=1, space="PSUM"))

    if preload is not None:
        w1T, w2t, b1tt, b2t = preload
    else:
        w1T, w2t, b1tt = [], [], []
        b2t = pool.tile([128, OUT], F32)
        nc.sync.dma_start(b2t[:], b2_d[:])
        for i, (k0, kn) in enumerate(KC):
            tw = pool.tile([128, D], F32, tag=f"w1{i}")
            nc.scalar.dma_start(tw[:kn, :], w1T_d[k0:k0 + kn, :])
            w1T.append(tw)
            t2 = pool.tile([128, OUT], F32, tag=f"w2{i}")
            nc.scalar.dma_start(t2[:kn, :], w2T_d[k0:k0 + kn, :])
            w2t.append(t2)
            tb = pool.tile([128, 1], F32, tag=f"b1{i}")
            nc.scalar.dma_start(tb[:kn, :], b1_d[k0:k0 + kn, :])
            b1tt.append(tb)

    dT = []
    for i, (k0, kn) in enumerate(KC):
        tg = pool.tile([128, D], F32, tag=f"dg{i}")
        nc.sync.dma_start(tg[:kn, :], dG_d[k0:k0 + kn, :])
        tp = pool.tile([128, D], F32, tag=f"dp{i}")
        nc.scalar.dma_start(tp[:kn, :], dPE_d[k0:k0 + kn, :])
        nc.vector.tensor_tensor(out=tg[:kn, :], in0=tg[:kn, :],
                                in1=tp[:kn, :], op=mybir.AluOpType.add)
        dT.append(tg)

    hT = []
    for jm, (j0, jn) in enumerate(KC):
        ph = psum.tile([128, D], F32, tag=f"ph{jm}", space="PSUM")
        for kc, (k0, kn) in enumerate(KC):
            nc.tensor.matmul(
                out=ph[:jn, :],
                lhsT=w1T[kc][:kn, j0:j0 + jn],
                rhs=dT[kc][:kn, :],
                start=(kc == 0),
                stop=(kc == 2),
            )
        th = pool.tile([128, D], F32, tag=f"h{jm}")
        nc.scalar.activation(th[:jn, :], ph[:jn, :],
                             mybir.ActivationFunctionType.Relu,
                             bias=b1tt[jm][:jn, :], scale=1.0)
        hT.append(th)

    for em, (e0, en) in enumerate(KC):
        pl = psum.tile([128, OUT], F32, tag=f"pl{em}", space="PSUM")
        for jm, (j0, jn) in enumerate(KC):
            nc.tensor.matmul(
                out=pl[:en, :],
                lhsT=hT[jm][:jn, e0:e0 + en],
                rhs=w2t[jm][:jn, :],
                start=(jm == 0),
                stop=(jm == 2),
            )
        lg = pool.tile([128, OUT], F32, tag=f"lg{em}")
        nc.vector.tensor_tensor(out=lg[:en, :], in0=pl[:en, :],
                                in1=b2t[:en, :], op=mybir.AluOpType.add)
        rmax = pool.tile([128, 1], F32, tag=f"rm{em}")
        nc.vector.reduce_max(rmax[:en, :], lg[:en, :],
                             axis=mybir.AxisListType.X)
        nmax = pool.tile([128, 1], F32, tag=f"nm{em}")
        nc.vector.tensor_scalar_mul(nmax[:en, :], rmax[:en, :], -1.0)
        ex = pool.tile([128, OUT], F32, tag=f"ex{em}")
        nc.scalar.activation(ex[:en, :], lg[:en, :],
                             mybir.ActivationFunctionType.Exp,
                             bias=nmax[:en, :], scale=1.0)
        ssum = pool.tile([128, 1], F32, tag=f"ss{em}")
        nc.vector.reduce_sum(ssum[:en, :], ex[:en, :],
                             axis=mybir.AxisListType.X)
        rcp = pool.tile([128, 1], F32, tag=f"rc{em}")
        nc.vector.reciprocal(rcp[:en, :], ssum[:en, :])
        so = pool.tile([128, OUT], F32, tag=f"so{em}")
        nc.vector.tensor_scalar_mul(so[:en, :], ex[:en, :], rcp[:en, :])
        nc.sync.dma_start(out_d[e0:e0 + en, :], so[:en, :])


# ---------------------------------------------------------------- phase 2

EC = 38   # e-columns of the head computed per core (8*38 = 304 >= 300)


def _build_phase2s():
    """e-sharded head: every core gets the full diag rows but only its own
    38-column e-slice; computes [38, 4] output rows.  The k/j dimension is
    zero-padded to 384 = 3*128 on the host so each tensor loads with a single
    DMA and all matmul chunks are uniform (zero rows contribute nothing, and
    hT's padded rows are relu(0 + 0) = 0)."""
    DP = 384
    nc = bacc.Bacc("TRN2", target_bir_lowering=False, debug=False,
                   num_devices=NCORES)

    dS_d = nc.dram_tensor("dS", [2 * DP, EC], F32, kind="ExternalInput").ap()
    w1T_d = nc.dram_tensor("w1Tp", [DP, D], F32, kind="ExternalInput").ap()
    b1_d = nc.dram_tensor("b1p", [DP, 1], F32, kind="ExternalInput").ap()
    w2T_d = nc.dram_tensor("w2Tp", [DP, OUT], F32, kind="ExternalInput").ap()
    b2_d = nc.dram_tensor("b2b", [128, OUT], F32, kind="ExternalInput").ap()
    out_d = nc.dram_tensor("out", [EC, OUT], F32, kind="ExternalOutput").ap()

    with tile.TileContext(nc) as tc, ExitStack() as ctx:
        pool = ctx.enter_context(tc.tile_pool(name="p2", bufs=1))
        psum = ctx.enter_context(tc.tile_pool(name="ps2", bufs=1, space="PSUM"))

        b2t = pool.tile([128, OUT], F32)
        nc.sync.dma_start(b2t[:], b2_d[:])
        tgp = pool.tile([128, 6 * EC], F32)
        nc.sync.dma_start(tgp[:].rearrange("p (c e) -> p c e", e=EC),
                          dS_d[:].rearrange("(c p) e -> p c e", p=128))
        nc.vector.tensor_tensor(out=tgp[:, :3 * EC], in0=tgp[:, :3 * EC],
                                in1=tgp[:, 3 * EC:], op=mybir.AluOpType.add)
        dT = [tgp[:, i * EC:(i + 1) * EC] for i in range(3)]
        w1t = pool.tile([128, 3 * D], F32)
        nc.sync.dma_start(w1t[:].rearrange("p (c j) -> p c j", j=D),
                          w1T_d[:].rearrange("(c p) j -> p c j", p=128))
        w1T = [w1t[:, i * D:(i + 1) * D] for i in range(3)]
        w2tt = pool.tile([128, 3 * OUT], F32)
        nc.scalar.dma_start(w2tt[:].rearrange("p (c o) -> p c o", o=OUT),
                            w2T_d[:].rearrange("(c p) o -> p c o", p=128))
        w2t = [w2tt[:, i * OUT:(i + 1) * OUT] for i in range(3)]
        b1t = pool.tile([128, 3], F32)
        nc.scalar.dma_start(b1t[:].rearrange("p (c x) -> p c x", x=1),
                            b1_d[:].rearrange("(c p) x -> p c x", p=128))

        # hT[j, e'] = relu(sum_k w1T[k, j] dT[k, e'] + b1[j])
        # j runs 0..299: chunks of (128, 128, 44); k contraction is 3x128
        # (padded k rows are zero and contribute nothing)
        JC = [(0, 128), (128, 128), (256, 44)]
        hT = []
        for jm, (j0, jn) in enumerate(JC):
            ph = psum.tile([128, EC], F32, tag=f"ph{jm}", space="PSUM")
            for kc in range(3):
                nc.tensor.matmul(
                    out=ph[:jn, :],
                    lhsT=w1T[kc][:, j0:j0 + jn],
                    rhs=dT[kc],
                    start=(kc == 0), stop=(kc == 2))
            th = pool.tile([128, EC], F32, tag=f"h{jm}")
            nc.scalar.activation(th[:jn, :], ph[:jn, :],
                                 mybir.ActivationFunctionType.Relu,
                                 bias=b1t[:jn, jm:jm + 1], scale=1.0)
            hT.append(th)

        # logits[e', o] = sum_j hT[j, e'] w2T[j, o] + b2[o]
        pl = psum.tile([128, OUT], F32, tag="pl", space="PSUM")
        for jm, (j0, jn) in enumerate(JC):
            nc.tensor.matmul(
                out=pl[:EC, :],
                lhsT=hT[jm][:jn, :],
                rhs=w2t[jm][:jn, :],
                start=(jm == 0), stop=(jm == 2))
        lg = pool.tile([128, OUT], F32, tag="lg")
        nc.vector.tensor_tensor(out=lg[:EC, :], in0=pl[:EC, :],
                                in1=b2t[:EC, :], op=mybir.AluOpType.add)
        rmax = pool.tile([128, 1], F32, tag="rm")
        nc.vector.reduce_max(rmax[:EC, :], lg[:EC, :], axis=mybir.AxisListType.X)
        nmax = pool.tile([128, 1], F32, tag="nm")
        nc.vector.tensor_scalar_mul(nmax[:EC, :], rmax[:EC, :], -1.0)
        ex = pool.tile([128, OUT], F32, tag="ex")
        nc.scalar.activation(ex[:EC, :], lg[:EC, :],
                             mybir.ActivationFunctionType.Exp,
                             bias=nmax[:EC, :], scale=1.0)
        ssum = pool.tile([128, 1], F32, tag="ss")
        nc.vector.reduce_sum(ssum[:EC, :], ex[:EC, :], axis=mybir.AxisListType.X)
        rcp = pool.tile([128, 1], F32, tag="rc")
        nc.vector.reciprocal(rcp[:EC, :], ssum[:EC, :])
        so = pool.tile([128, OUT], F32, tag="so")
        nc.vector.tensor_scalar_mul(so[:EC, :], ex[:EC, :], rcp[:EC, :])
        nc.sync.dma_start(out_d[:], so[:EC, :])

    nc.compile()
    return nc


_CACHE = {}
# Fused (single-launch, AllGather) variant exists but costs ~2x15us of
# collective fixed overhead; the two-launch variant is faster on device time.
FUSED = False


def _phase1(fused=False):
    key = "pf" if fused else "p1"
    if key not in _CACHE:
        _CACHE[key] = _build_phase1(fused=fused)
    return _CACHE[key]


def _phase2s():
    if "p2s" not in _CACHE:
        _CACHE["p2s"] = _build_phase2s()
    return _CACHE["p2s"]


# ---------------------------------------------------------------- host glue

def _pe_table():
    pos = np.arange(L, dtype=np.float32)[:, None]
    div = np.exp(np.arange(0, D, 2, dtype=np.float32)
                 * np.float32(-np.log(10000.0) / D))
    pe = np.zeros((L, D), dtype=np.float32)
    pe[:, 0::2] = np.sin(pos * div)
    pe[:, 1::2] = np.cos(pos * div)
    return pe


def _wrap_idx(rows):
    """rows [nk, 512] -> int16 [128, nk*32] in dma_gather's wrapped layout
    (per CHUNK_SIZES blocks; idx i of a chunk sits at [i%16, blockcol+i//16],
    replicated down all 128 partitions)."""
    out = np.zeros((16, rows.shape[0] * 32), dtype=np.int16)
    off = 0
    for ch in CHUNK_SIZES:
        seq = rows[off:off + ch].reshape(-1)            # ch*512
        out[:, off * 32:(off + ch) * 32] = seq.reshape(-1, 16).T
        off += ch
    return np.tile(out, (8, 1))


def kernel(x1, x2, emb1, emb2, w1, b1, w2, b2, _trace=(False, False)):
    x1 = np.asarray(x1); x2 = np.asarray(x2)
    emb1 = np.ascontiguousarray(np.asarray(emb1, dtype=np.float32))
    emb2 = np.ascontiguousarray(np.asarray(emb2, dtype=np.float32))
    w1 = np.asarray(w1, dtype=np.float32); b1 = np.asarray(b1, dtype=np.float32)
    w2 = np.asarray(w2, dtype=np.float32); b2 = np.asarray(b2, dtype=np.float32)

    pe = _pe_table()
    emb1p = np.zeros((V, EP), dtype=np.float32)
    emb1p[:, :D] = emb1

    # pe4: [p, c*300+e] = pe[c*128+p, e]
    pe4 = np.ascontiguousarray(
        pe.reshape(4, 128, D).transpose(1, 0, 2).reshape(128, 4 * D))

    DP = 384
    w1Tp = np.zeros((DP, D), dtype=np.float32)
    w1Tp[:D] = w1.T
    b1p = np.zeros((DP, 1), dtype=np.float32)
    b1p[:D, 0] = b1
    w2Tp = np.zeros((DP, OUT), dtype=np.float32)
    w2Tp[:D] = w2.T
    b2b = np.ascontiguousarray(np.tile(b2.reshape(1, OUT), (128, 1)))

    in_maps = []
    for core in range(NCORES):
        k0 = NK * core
        kidx = np.arange(k0, k0 + NK)
        x1w = _wrap_idx(x1[k0:k0 + NK].astype(np.int64))
        x2w = _wrap_idx(x2[k0:k0 + NK].astype(np.int64))
        nch = min(NK, max(0, D - k0))        # real channels for this core
        emb2sl = np.zeros((V, E2P), dtype=np.float32)
        emb2sl[:, :nch] = emb2[:, k0:k0 + nch]
        # pe_cols[p, kk*4+c] = pe[c*128+p, k0+kk] (0 when k >= 300)
        pec = np.zeros((128, NK * 4), dtype=np.float32)
        valid = kidx < D
        pev = pe[:, kidx[valid]].reshape(4, 128, valid.sum())  # [c, p, kk]
        pec_v = pec.reshape(128, NK, 4)
        pec_v[:, valid, :] = pev.transpose(1, 2, 0)
        im = {
            "emb1p": emb1p,
            "emb2sl": emb2sl,
            "x1w": x1w,
            "x2w": x2w,
            "pe4": pe4,
            "pec": pec,
        }
        if FUSED:
            im.update({"w1T": np.ascontiguousarray(w1Tp[:D]),
                       "b1c": np.ascontiguousarray(b1p[:D]),
                       "w2T": np.ascontiguousarray(w2Tp[:D]),
                       "b2b": b2b})
        in_maps.append(im)

    if FUSED:
        res1 = run_bass_kernel_spmd(_phase1(fused=True), in_maps,
                                    core_ids=list(range(NCORES)),
                                    trace=_trace[0])
        out = res1.results[0]["out"]
        if _trace[0]:
            kernel._last_exec_ns = (res1.exec_time_ns, None)
            kernel._last_results = (res1, None)
        return out

    res1 = run_bass_kernel_spmd(_phase1(), in_maps,
                                core_ids=list(range(NCORES)), trace=_trace[0])
    diagG = np.concatenate(
        [r["diagG"].reshape(NK, D) for r in res1.results])[:D]
    diagPE = np.concatenate(
        [r["diagPE"] for r in res1.results])[:D]

    # e-sharded head: every core gets the full k-rows but only its own
    # 38-wide e-column slice of the diagonal
    in2_maps = []
    for core in range(NCORES):
        e0 = EC * core
        ne = min(EC, max(0, D - e0))
        dS = np.zeros((2 * DP, EC), dtype=np.float32)
        dS[:D, :ne] = diagG[:, e0:e0 + ne]
        dS[DP:DP + D, :ne] = diagPE[:, e0:e0 + ne]
        in2_maps.append({
            "dS": dS,
            "w1Tp": w1Tp,
            "b1p": b1p,
            "w2Tp": w2Tp,
            "b2b": b2b,
        })
    res2 = run_bass_kernel_spmd(_phase2s(), in2_maps,
                                core_ids=list(range(NCORES)), trace=_trace[1])
    out = np.concatenate([r["out"] for r in res2.results])[:D]

    if _trace[0] or _trace[1]:
        kernel._last_exec_ns = (res1.exec_time_ns, res2.exec_time_ns)
        kernel._last_results = (res1, res2)
    return out
